# revision 48
# baseline (speedup 1.0000x reference)
"""Trainium2 Bass kernel for nn_AdExternal_N3Tree (gnn_message_passing).

Key insight: the reference's sequential 32768-step scan only affects the
output through `acc` (a 32-vector). Each parent's 8-child sibling group is an
independent serial chain that is LINEAR in that parent's original 8x32 block,
and group depth is constant within each of 6 contiguous parent-index classes.
So  acc = sum_d M_d @ s_d + gamma_tot,  where s_d is the sum of flattened
parent blocks over class d (a pure 4MB reduction) and M_d (32x256) / gamma
are tiny matrices computed on the host from conv_w/conv_b/depth_weight alone.

The leaf cells that feed the MLPs (flat cells 32767..262143) are never
written by the scan, so  out[leaf] = MLP(data_flat[leaf] + acc)  and cells
0..32766 are zero.

Device work per core (SPMD on 8 cores, no collectives - each core is fully
independent, which also makes the kernel immune to cross-core launch skew):
  - scan DMA as 9 contiguous-DRAM slices interleaved over the sync + gpsimd
    queues (per-slice DRAM tensors double effective HBM bandwidth vs column
    slices of one tensor; the last slice is a single 64KB tile so the bias
    critical path ends on a small, early transfer); consts ride the scalar
    queue ONLY in small volume (bulk scalar-queue DMA provably drops the
    whole-chip clock by 1.2x); xT is dep-blocked behind the scan except a
    small chunk-0-2 piece that prefills z
  - stage 1 pipelined per slice: class sums with two PE quadrants: mixed
    node tiles cycle indicator weights in quadrant 0; the 26 pure-class-4
    tiles reuse ONE resident indicator in quadrant 1 (LDWEIGHTS elision ->
    256-col feed-bound, 213ns/tile)
  - tiny chain in bf16: s -> transpose (R-matrix matmuls fold both quadrant
    row groups) -> even/odd acc quadrants -> widened [65,128] bias matmul
    merges them for free -> folded layer-1 bias (128,)
  - MLP over a 28680-cell slice in bf16: x@W1cat (row-tiled) -> GELU+bias on
    ScalarE (the 1 elem/cycle/lane throughput bound, f32 PSUM in, 1.44us per
    1536-cell chunk) -> @W2cat (col-tiled) -> +b2 evict on DVE; the chain's
    single PSUM bank + outer z-pool lets z0/z1 prefill during the chain
  - output in 5 staged gpsimd batches; the final 512-col batch fans one
    strip to each of the 3 queues so completion latencies overlap.
"""
import sys

for _p in ("/opt/trn_rl_repo", "/opt/trn_rl_repo/concourse"):
    if _p not in sys.path:
        sys.path.insert(0, _p)

import numpy as np

N_NODES = 32768
D = 32
N_GROUPS = 4096
N_CELLS = N_NODES * 8           # 262144
LEAF0 = N_NODES - 1             # 32767 first leaf cell
CORES = 8
CELLS_MAIN = 28672              # cells each core contributes (core 7: +1)
CH = 1536                       # cells per chunk (3 row-tiled sub-chunks of 512)
NCH = 19
SUB = 512
NSUB = 3
SUB_L = 512                     # last-chunk sub size (PSUM writes must stay
                                # bank-aligned, so no odd-size trim)
CH_L = NSUB * SUB_L
CELLS_CORE = (NCH - 1) * CH + CH_L   # 28680 >= 28673
SCAN_TILES = 32                 # replicated scan: 32 node-tiles of 128
SCAN_FREE = SCAN_TILES * 256    # 8192
XT_FREE = (NCH - 1) * SUB + SUB_L    # 9560 (free cols per band row)

# stage-1 tile classification: tiles fully inside class 4 share one
# indicator (nodes 640..3967 = tiles 5..30); the rest are "mixed"
PURE_LO, PURE_HI = 5, 30

# scan DMA slice boundaries in 128-node tiles: the LAST slice is a single
# tile so the bias critical path ends on a small, early-landing transfer
SCAN_CUTS = [0, 8, 16, 24, 31, 32]

# constsB (bf16, [128, NB]) column map (ind lives in its own fp8 tensor)
R0 = 0                          # R fold matrix [38, 6]
M20 = R0 + 6                    # 6: m2 [128, 12*32]
WB0 = M20 + 384                 # 390: wb2 [65, 128]
W10 = WB0 + 128                 # 518: w1cat3 [96, 128]
W20 = W10 + 128                 # 646: w2 [128, 4]
NB = W20 + 4                    # 650

# (p_lo, p_hi_inclusive, conv_depth, n_children, extra_j0_step)
CLASSES = [
    (0, 0, 1, 8, True),
    (1, 8, 2, 8, False),
    (9, 72, 3, 8, False),
    (73, 584, 4, 8, False),
    (585, 4094, 5, 8, False),
    (4095, 4095, 5, 7, False),
]


# ---------------------------------------------------------------- host math
def _chain(conv_w_d, conv_b_d, dw_d, n_children):
    W = conv_w_d.astype(np.float64)
    b = conv_b_d.astype(np.float64)
    Wk = [W[:, :, k] for k in range(8)]
    A, beta = {}, {}
    if n_children == 7:
        A7 = np.zeros((8, D, D))
        A7[7] = np.eye(D)
        A[7] = A7
        beta[7] = np.zeros(D)
        cs = range(6, -1, -1)
    else:
        cs = range(7, -1, -1)
    for c in cs:
        Ac = np.zeros((8, D, D))
        bc = b.copy()
        for k in range(0, c + 1):
            Ac[k] += Wk[k]
        for m in range(c + 1, 8):
            for k in range(8):
                Ac[k] += Wk[m] @ A[m][k]
            bc += Wk[m] @ beta[m]
        A[c] = Ac
        beta[c] = bc
    Msum = np.zeros((8, D, D))
    gamma = np.zeros(D)
    for c in (range(8) if n_children == 8 else range(7)):
        Msum += dw_d * A[c]
        gamma += dw_d * beta[c]
    return A, beta, Msum, gamma


def _build_class_mats(conv_w, conv_b, depth_weight):
    out = []
    for (p_lo, p_hi, dep, nch, extra) in CLASSES:
        A, beta, Msum, gamma = _chain(
            conv_w[dep], conv_b[dep], float(depth_weight[dep]), nch
        )
        if extra:
            W0 = conv_w[0].astype(np.float64)
            b0 = conv_b[0].astype(np.float64)
            W0k = [W0[:, :, k] for k in range(8)]
            Ae = np.zeros((8, D, D))
            be = b0.copy()
            for m in range(8):
                for k in range(8):
                    Ae[k] += W0k[m] @ A[m][k]
                be += W0k[m] @ beta[m]
            Msum = Msum + float(depth_weight[0]) * Ae
            gamma = gamma + float(depth_weight[0]) * be
        M = np.concatenate([Msum[k] for k in range(8)], axis=1)  # (D, 8D)
        out.append((p_lo, p_hi, M, gamma))
    return out


# ---------------------------------------------------------------- device graph
_GRAPH = None


def _build_graph():
    import concourse.bacc as bacc
    import concourse.mybir as mybir
    from concourse import tile
    from concourse.tile_rust import add_dep_helper

    F32 = mybir.dt.float32
    BF16 = mybir.dt.bfloat16
    nc = bacc.Bacc("TRN2", target_bir_lowering=False, debug=False, num_devices=CORES)

    cb_d = nc.declare_dram_parameter("cb", [128, NB], BF16, isOutput=False)
    ind_d = nc.declare_dram_parameter("ind8", [128, 200], BF16, isOutput=False)
    # scan/xT split into separate DRAM tensors so each transfer reads
    # CONTIGUOUS DRAM (a column-slice of one big tensor is 16KB-strided
    # 2KB chunks, which halves effective HBM bandwidth)
    scan_ds = [
        nc.declare_dram_parameter(
            f"scan{k}",
            [128, 256 * (SCAN_CUTS[k + 1] - SCAN_CUTS[k])], BF16,
            isOutput=False,
        )
        for k in range(len(SCAN_CUTS) - 1)
    ]
    XT_BOUNDS = [0, 1536, 4096, 6656, XT_FREE]
    xT_ds = [
        nc.declare_dram_parameter(
            f"xT{q}", [96, XT_BOUNDS[q + 1] - XT_BOUNDS[q]], BF16, isOutput=False
        )
        for q in range(4)
    ]
    b2_d = nc.declare_dram_parameter("b2col", [128, 1], F32, isOutput=False)
    out_d = nc.declare_dram_parameter("out", [12, XT_FREE], F32, isOutput=True)

    Gelu = mybir.ActivationFunctionType.Gelu

    with tile.TileContext(nc) as tc:
        with (
            tc.tile_pool(name="const", bufs=1) as cpool,
            tc.tile_pool(name="data", bufs=1) as dpool,
            tc.tile_pool(name="gp", bufs=3) as gpool,
        ):
            warm_sb = cpool.tile([1, 8], F32)
            warmd_sb = cpool.tile([32, 128], BF16)
            cb_sb = cpool.tile([128, NB], BF16)
            ind_sb = cpool.tile([128, 200], BF16)
            b2_sb = cpool.tile([128, 1], F32)
            acc1 = cpool.tile([65, 1], BF16)
            bias_sb = cpool.tile([128, 1], F32)
            s_sb = cpool.tile([38, 256], BF16)
            sT_sb = cpool.tile([128, 12], BF16)

            scan_sb = dpool.tile([128, SCAN_FREE], BF16)
            xT_sb = dpool.tile([96, XT_FREE], BF16)
            stage_sb = dpool.tile([128, XT_FREE], F32)

            # ---- DMA enqueues ----
            # ind + consts first on scalar (needed by stage 1); fp8 scan
            # slices interleave across sync/gpsimd in consumption order; xT
            # quarters wait for the whole scan (keeps the bias critical path
            # at full DMA bandwidth); b2col last (needed at ~first add)
            nc.scalar.dma_start(ind_sb[:], ind_d.ap())
            nc.scalar.dma_start(cb_sb[:], cb_d.ap())
            SLICE_Q = [nc.sync, nc.gpsimd, nc.sync, nc.gpsimd, nc.sync]
            scan_dmas = []
            for k, eng in enumerate(SLICE_Q):
                lo, hi = 256 * SCAN_CUTS[k], 256 * SCAN_CUTS[k + 1]
                scan_dmas.append(eng.dma_start(
                    scan_sb[:, lo:hi], scan_ds[k].ap(),
                ))
            # xT: a small early piece (chunks 0-2) rides gpsimd with no dep
            # so z0-z2 can prefill during the chain; the remaining three
            # pieces wait for the scan (bias critical path owns the HBM)
            XT_PIECE_Q = [nc.gpsimd, nc.sync, nc.gpsimd, nc.sync]
            for q, eng in enumerate(XT_PIECE_Q):
                lo, hi = XT_BOUNDS[q], XT_BOUNDS[q + 1]
                xi = eng.dma_start(xT_sb[:, lo:hi], xT_ds[q].ap())
                if q > 0:
                    for sd in scan_dmas:
                        add_dep_helper(xi.ins, sd.ins, sync=True,
                                       reason="serialize xT behind scan")
            nc.scalar.dma_start(b2_sb[:], b2_d.ap())

            # warm-ups AFTER the dma enqueues so the queues start moving
            # first: ACT warm (gelu table load) + PE warm-up source
            nc.gpsimd.memset(warm_sb[:], 0.0)
            nc.scalar.activation(warm_sb[:], warm_sb[:], Gelu)
            nc.gpsimd.memset(warmd_sb[:], 0.001)
            nc.gpsimd.memset(acc1[64:65, :], 1.0)

            with tc.tile_pool(name="psZ", bufs=2, space="PSUM") as zp:
                with tc.tile_pool(name="psC", bufs=1, space="PSUM") as pchain:
                    # chain PSUM lives in ONE bank: cols 0:256 stage-1
                    # class sums (+ warm-up junk), 256:268 sT(A+B),
                    # 268:280 sT(C), 280:281 acc E/O, 281:282 bias
                    ps_part = pchain.tile([128, 284], F32)

                    # PE pre-warm: open the HAM clock gate before stage 1
                    for _ in range(16):
                        nc.tensor.matmul(
                            ps_part[:, 0:128], warmd_sb[:], warmd_sb[:],
                            start=True, stop=True,
                        )

                    # stage 1: class sums over the replicated scan region.
                    # mixed tiles cycle indicators in quadrant 0 (rows 0-5);
                    # pure class-4 tiles share tile-5's indicator resident in
                    # quadrant 1 (rows 32-37) -> no LDWEIGHTS between them
                    for T in range(SCAN_TILES):
                        pure = PURE_LO <= T <= PURE_HI
                        ind_T = 6 * PURE_LO if pure else 6 * T
                        nc.tensor.matmul(
                            ps_part[32:38, 0:256] if pure else ps_part[0:6, 0:256],
                            ind_sb[:, ind_T:ind_T + 6],
                            scan_sb[:, 256 * T:256 * (T + 1)],
                            start=(T == PURE_LO if pure else T == 0),
                            stop=(T == PURE_HI if pure else T == SCAN_TILES - 1),
                            tile_position=(0, 32) if pure else (0, 0),
                        )

                    # s (38,256) -> sT (128,12) via R-matmuls that also fold
                    # the two quadrant row groups (R[d,d]=R[32+d,d]=1)
                    nc.vector.tensor_copy(s_sb[:], ps_part[0:38, 0:256])
                    for jhi in range(2):
                        nc.tensor.matmul(
                            ps_part[:, 256 + 6 * jhi:256 + 6 * jhi + 6],
                            s_sb[:, 128 * jhi:128 * (jhi + 1)],
                            cb_sb[0:38, R0:R0 + 6],
                            start=True, stop=True,
                        )
                    nc.vector.tensor_copy(sT_sb[:], ps_part[:, 256:268])

                    # acc = sum_k M2_k @ sT[:, k], even k in quadrant 0
                    # (rows 0:32), odd k in quadrant 1 (rows 32:64)
                    for k in range(12):
                        odd = k % 2
                        nc.tensor.matmul(
                            ps_part[32 * odd:32 * odd + 32, 280:281],
                            cb_sb[:, M20 + 32 * k:M20 + 32 * (k + 1)],
                            sT_sb[:, k:k + 1],
                            start=(k < 2), stop=(k >= 10),
                            tile_position=(0, 32 * odd),
                        )
                    nc.vector.tensor_copy(acc1[0:64, :], ps_part[0:64, 280:281])

                    # bias1_eff = W1cat.T@(accE+accO) + (b1cat + gamma@W1cat)
                    # via the widened [65,128] wb2 (rows 0-31 W1, 32-63 W1,
                    # 64 bconst)
                    nc.tensor.matmul(
                        ps_part[:, 281:282], cb_sb[0:65, WB0:WB0 + 128],
                        acc1[:], start=True, stop=True,
                    )
                    nc.vector.tensor_copy(bias_sb[:], ps_part[:, 281:282])

                with tc.tile_pool(name="psO", bufs=2, space="PSUM") as op:
                    for t in range(NCH):
                        sub = SUB if t < NCH - 1 else SUB_L
                        ch = NSUB * sub
                        t0 = SUB * t
                        z = zp.tile([128, CH], F32)
                        for a in range(NSUB):
                            nc.tensor.matmul(
                                z[:, sub * a:sub * (a + 1)],
                                cb_sb[32 * a:32 * (a + 1), W10:W10 + 128],
                                xT_sb[32 * a:32 * (a + 1), t0:t0 + sub],
                                start=True,
                                stop=True,
                                tile_position=(32 * a, 0),
                            )
                        g = gpool.tile([128, CH], BF16)
                        nc.scalar.activation(g[:, 0:ch], z[:, 0:ch], Gelu,
                                             bias=bias_sb[:])
                        o_ps = op.tile([128, SUB], F32)
                        for c in range(NSUB):
                            nc.tensor.matmul(
                                o_ps[32 * c:32 * c + 4, 0:sub],
                                cb_sb[:, W20:W20 + 4],
                                g[:, sub * c:sub * (c + 1)],
                                start=True,
                                stop=True,
                                tile_position=(0, 32 * c),
                            )
                        nc.vector.tensor_scalar_add(
                            stage_sb[:, t0:t0 + sub], o_ps[:, 0:sub], b2_sb[:]
                        )
                        # batched output DMA on the idle gpsimd queue; the
                        # final (small) batch fans one strip to each of the
                        # 3 queues so their completion latencies overlap
                        if t in (4, 9, 13, 16, 17, NCH - 1):
                            lo = {4: 0, 9: 2560, 13: 5120, 16: 7168,
                                  17: 8704, NCH - 1: 9216}[t]
                            hi = t0 + sub
                            engs = ([nc.gpsimd] * 3 if t != NCH - 1
                                    else [nc.sync, nc.gpsimd, nc.scalar])
                            for c in range(NSUB):
                                engs[c].dma_start(
                                    out_d.ap()[4 * c:4 * c + 4, lo:hi],
                                    stage_sb[32 * c:32 * c + 4, lo:hi],
                                )

    nc.compile()
    return nc


def _get_graph():
    global _GRAPH
    if _GRAPH is None:
        _GRAPH = _build_graph()
    return _GRAPH


# ---------------------------------------------------------------- kernel
def kernel(**inputs):
    import ml_dtypes
    from concourse import bass_utils

    data = np.asarray(inputs["data"], np.float32)
    conv_w = np.asarray(inputs["conv_w"], np.float32)
    conv_b = np.asarray(inputs["conv_b"], np.float32)
    dw = np.asarray(inputs["depth_weight"], np.float32)
    f_w1 = np.asarray(inputs["f_w1"], np.float32)
    f_b1 = np.asarray(inputs["f_b1"], np.float32)
    f_w2 = np.asarray(inputs["f_w2"], np.float32)
    f_b2 = np.asarray(inputs["f_b2"], np.float32)
    s_w1 = np.asarray(inputs["s_w1"], np.float32)
    s_b1 = np.asarray(inputs["s_b1"], np.float32)
    s_w2 = np.asarray(inputs["s_w2"], np.float32)
    s_b2 = np.asarray(inputs["s_b2"], np.float32)

    # --- weight-derived host constants (no data-sized work here) ---
    mats = _build_class_mats(conv_w, conv_b, dw)

    W1cat = np.concatenate([f_w1, s_w1], axis=1)          # (32, 128)
    b1cat = np.concatenate([f_b1, s_b1])                  # (128,)
    gamma_tot = np.zeros(D)
    for (p_lo, p_hi, M, gamma) in mats:
        gamma_tot += (p_hi - p_lo + 1) * gamma
    bconst = b1cat.astype(np.float64) + gamma_tot @ W1cat.astype(np.float64)

    W2cat = np.zeros((128, 4), np.float32)
    W2cat[0:64, 0:3] = f_w2
    W2cat[64:128, 3:4] = s_w2
    b2cat = np.concatenate([f_b2, s_b2]).astype(np.float32)
    b2col = np.zeros((128, 1), np.float32)
    for c in range(NSUB):
        b2col[32 * c:32 * c + 4, 0] = b2cat

    # --- packed constants ---
    # ind (fp8): col block 6T+d, row p: node 128T+p in class d
    ind8 = np.zeros((128, 200), np.float32)
    for dcls, (p_lo, p_hi, M, gamma) in enumerate(mats):
        for node in range(p_lo, p_hi + 1):
            T, p = divmod(node, 128)
            ind8[p, 6 * T + dcls] = 1.0
    # cols 192:198 duplicate the pure-class-4 indicator at a distinct
    # address so group C's LDWEIGHTS (new PE quadrant) is not elided
    ind8[:, 192:198] = ind8[:, 6 * PURE_LO:6 * PURE_LO + 6]
    ind8 = np.ascontiguousarray(ind8.astype(ml_dtypes.bfloat16))

    cb = np.zeros((128, NB), np.float32)
    # R fold matrix: quadrant-0 rows 0-5 and quadrant-1 rows 32-37 -> class d
    for dcls in range(6):
        cb[dcls, R0 + dcls] = 1.0
        cb[32 + dcls, R0 + dcls] = 1.0
    # m2 (128, 384): col block k=6*jhi+d : m2[j, 32k+o] = M_d[o, 128*jhi+j]
    for dcls, (p_lo, p_hi, M, gamma) in enumerate(mats):
        Mf = M.astype(np.float32)
        for jhi in range(2):
            k = 6 * jhi + dcls
            cb[:, M20 + 32 * k:M20 + 32 * (k + 1)] = \
                Mf[:, 128 * jhi:128 * (jhi + 1)].T
    # wb2 (65, 128): rows 0-31 W1cat, 32-63 W1cat, 64 bconst
    cb[0:32, WB0:WB0 + 128] = W1cat
    cb[32:64, WB0:WB0 + 128] = W1cat
    cb[64, WB0:WB0 + 128] = bconst.astype(np.float32)
    # w1cat3 (96, 128) and w2 (128, 4)
    cb[0:96, W10:W10 + 128] = np.tile(W1cat, (3, 1))
    cb[:, W20:W20 + 4] = W2cat
    cb = np.ascontiguousarray(cb.astype(ml_dtypes.bfloat16))

    # --- shards ---
    data_flat = data.reshape(N_CELLS, D)

    # replicated scan region (all 4096 parent nodes), bf16, one contiguous
    # array per DMA slice
    scan = (
        data_flat[0:N_GROUPS * 8].reshape(SCAN_TILES, 128, 256).transpose(1, 0, 2)
        .reshape(128, SCAN_FREE).astype(ml_dtypes.bfloat16)
    )
    scan_slices = [
        np.ascontiguousarray(scan[:, 256 * SCAN_CUTS[k]:256 * SCAN_CUTS[k + 1]])
        for k in range(len(SCAN_CUTS) - 1)
    ]
    XT_BOUNDS = [0, 1536, 4096, 6656, XT_FREE]

    in_maps = []
    for i in range(CORES):
        base = LEAF0 + CELLS_MAIN * i
        end = min(base + CELLS_CORE, N_CELLS)
        x_lin = np.zeros((CELLS_CORE, D), np.float32)
        x_lin[0:end - base] = data_flat[base:end]
        xA = (
            x_lin[:(NCH - 1) * CH].reshape(NCH - 1, NSUB, SUB, D)
            .transpose(1, 3, 0, 2).reshape(96, (NCH - 1) * SUB)
        )
        xB = (
            x_lin[(NCH - 1) * CH:].reshape(NSUB, SUB_L, D)
            .transpose(0, 2, 1).reshape(96, SUB_L)
        )
        xT = np.concatenate([xA, xB], axis=1).astype(ml_dtypes.bfloat16)
        im = {"cb": cb, "ind8": ind8, "b2col": b2col}
        for k in range(len(SCAN_CUTS) - 1):
            im[f"scan{k}"] = scan_slices[k]
        for q in range(4):
            im[f"xT{q}"] = np.ascontiguousarray(
                xT[:, XT_BOUNDS[q]:XT_BOUNDS[q + 1]]
            )
        in_maps.append(im)

    nc = _get_graph()
    res = bass_utils.run_bass_kernel_spmd(nc, in_maps, core_ids=list(range(CORES)))

    out_flat = np.zeros((N_CELLS, 4), np.float32)
    for i in range(CORES):
        base = LEAF0 + CELLS_MAIN * i
        k = CELLS_MAIN if i < CORES - 1 else CELLS_MAIN + 1
        # planes (12, 9560): row 4c+o holds cells of chunk t at free 512t+cc
        planes = res.results[i]["out"]
        pA = planes[:, :(NCH - 1) * SUB].reshape(NSUB, 4, NCH - 1, SUB)
        cellsA = pA.transpose(2, 0, 3, 1).reshape((NCH - 1) * CH, 4)
        pB = planes[:, (NCH - 1) * SUB:].reshape(NSUB, 4, SUB_L)
        cellsB = pB.transpose(0, 2, 1).reshape(CH_L, 4)
        cells = np.concatenate([cellsA, cellsB], axis=0)   # (28680, 4)
        out_flat[base:base + k] = cells[:k]
    return out_flat.reshape(N_NODES, 2, 2, 2, 4)


# revision 49
# speedup vs baseline: 1.0237x; 1.0237x over previous
"""Trainium2 Bass kernel for nn_AdExternal_N3Tree (gnn_message_passing).

Key insight: the reference's sequential 32768-step scan only affects the
output through `acc` (a 32-vector). Each parent's 8-child sibling group is an
independent serial chain that is LINEAR in that parent's original 8x32 block,
and group depth is constant within each of 6 contiguous parent-index classes.
So  acc = sum_d M_d @ s_d + gamma_tot,  where s_d is the sum of flattened
parent blocks over class d (a pure 4MB reduction) and M_d (32x256) / gamma
are tiny matrices computed on the host from conv_w/conv_b/depth_weight alone.

The leaf cells that feed the MLPs (flat cells 32767..262143) are never
written by the scan, so  out[leaf] = MLP(data_flat[leaf] + acc)  and cells
0..32766 are zero.

Device work per core (SPMD on 8 cores, no collectives - each core is fully
independent, which also makes the kernel immune to cross-core launch skew):
  - scan DMA as 9 contiguous-DRAM slices interleaved over the sync + gpsimd
    queues (per-slice DRAM tensors double effective HBM bandwidth vs column
    slices of one tensor; the last slice is a single 64KB tile so the bias
    critical path ends on a small, early transfer); consts ride the scalar
    queue ONLY in small volume (bulk scalar-queue DMA provably drops the
    whole-chip clock by 1.2x); xT is dep-blocked behind the scan except a
    small chunk-0-2 piece that prefills z
  - stage 1 pipelined per slice: class sums with two PE quadrants: mixed
    node tiles cycle indicator weights in quadrant 0; the 26 pure-class-4
    tiles reuse ONE resident indicator in quadrant 1 (LDWEIGHTS elision ->
    256-col feed-bound, 213ns/tile)
  - tiny chain in bf16: s -> transpose (R-matrix matmuls fold both quadrant
    row groups) -> even/odd acc quadrants -> widened [65,128] bias matmul
    merges them for free -> folded layer-1 bias (128,)
  - MLP over a 28680-cell slice in bf16: x@W1cat (row-tiled) -> GELU+bias on
    ScalarE (the 1 elem/cycle/lane throughput bound, f32 PSUM in, 1.44us per
    1536-cell chunk) -> @W2cat (col-tiled) -> +b2 evict on DVE; the chain's
    single PSUM bank + outer z-pool lets z0/z1 prefill during the chain
  - output in 5 staged gpsimd batches; the final 512-col batch fans one
    strip to each of the 3 queues so completion latencies overlap.
"""
import sys

for _p in ("/opt/trn_rl_repo", "/opt/trn_rl_repo/concourse"):
    if _p not in sys.path:
        sys.path.insert(0, _p)

import numpy as np

N_NODES = 32768
D = 32
N_GROUPS = 4096
N_CELLS = N_NODES * 8           # 262144
LEAF0 = N_NODES - 1             # 32767 first leaf cell
CORES = 8
CELLS_MAIN = 28672              # cells each core contributes (core 7: +1)
CH = 1536                       # cells per chunk (3 row-tiled sub-chunks of 512)
NCH = 19
SUB = 512
NSUB = 3
SUB_L = 512                     # last-chunk sub size (PSUM writes must stay
                                # bank-aligned, so no odd-size trim)
CH_L = NSUB * SUB_L
CELLS_CORE = (NCH - 1) * CH + CH_L   # 28680 >= 28673
SCAN_TILES = 32                 # replicated scan: 32 node-tiles of 128
SCAN_FREE = SCAN_TILES * 256    # 8192
XT_FREE = (NCH - 1) * SUB + SUB_L    # 9560 (free cols per band row)

# stage-1 tile classification: tiles fully inside class 4 share one
# indicator (nodes 640..3967 = tiles 5..30); the rest are "mixed"
PURE_LO, PURE_HI = 5, 30

# scan DMA slice boundaries in 128-node tiles: the LAST slice is a single
# tile so the bias critical path ends on a small, early-landing transfer
SCAN_CUTS = [0, 4, 8, 12, 16, 20, 24, 28, 31, 32]

# constsB (bf16, [128, NB]) column map (ind lives in its own fp8 tensor)
R0 = 0                          # R fold matrix [38, 6]
M20 = R0 + 6                    # 6: m2 [128, 12*32]
WB0 = M20 + 384                 # 390: wb2 [65, 128]
W10 = WB0 + 128                 # 518: w1cat3 [96, 128]
W20 = W10 + 128                 # 646: w2 [128, 4]
NB = W20 + 4                    # 650

# (p_lo, p_hi_inclusive, conv_depth, n_children, extra_j0_step)
CLASSES = [
    (0, 0, 1, 8, True),
    (1, 8, 2, 8, False),
    (9, 72, 3, 8, False),
    (73, 584, 4, 8, False),
    (585, 4094, 5, 8, False),
    (4095, 4095, 5, 7, False),
]


# ---------------------------------------------------------------- host math
def _chain(conv_w_d, conv_b_d, dw_d, n_children):
    W = conv_w_d.astype(np.float64)
    b = conv_b_d.astype(np.float64)
    Wk = [W[:, :, k] for k in range(8)]
    A, beta = {}, {}
    if n_children == 7:
        A7 = np.zeros((8, D, D))
        A7[7] = np.eye(D)
        A[7] = A7
        beta[7] = np.zeros(D)
        cs = range(6, -1, -1)
    else:
        cs = range(7, -1, -1)
    for c in cs:
        Ac = np.zeros((8, D, D))
        bc = b.copy()
        for k in range(0, c + 1):
            Ac[k] += Wk[k]
        for m in range(c + 1, 8):
            for k in range(8):
                Ac[k] += Wk[m] @ A[m][k]
            bc += Wk[m] @ beta[m]
        A[c] = Ac
        beta[c] = bc
    Msum = np.zeros((8, D, D))
    gamma = np.zeros(D)
    for c in (range(8) if n_children == 8 else range(7)):
        Msum += dw_d * A[c]
        gamma += dw_d * beta[c]
    return A, beta, Msum, gamma


def _build_class_mats(conv_w, conv_b, depth_weight):
    out = []
    for (p_lo, p_hi, dep, nch, extra) in CLASSES:
        A, beta, Msum, gamma = _chain(
            conv_w[dep], conv_b[dep], float(depth_weight[dep]), nch
        )
        if extra:
            W0 = conv_w[0].astype(np.float64)
            b0 = conv_b[0].astype(np.float64)
            W0k = [W0[:, :, k] for k in range(8)]
            Ae = np.zeros((8, D, D))
            be = b0.copy()
            for m in range(8):
                for k in range(8):
                    Ae[k] += W0k[m] @ A[m][k]
                be += W0k[m] @ beta[m]
            Msum = Msum + float(depth_weight[0]) * Ae
            gamma = gamma + float(depth_weight[0]) * be
        M = np.concatenate([Msum[k] for k in range(8)], axis=1)  # (D, 8D)
        out.append((p_lo, p_hi, M, gamma))
    return out


# ---------------------------------------------------------------- device graph
_GRAPH = None


def _build_graph():
    import concourse.bacc as bacc
    import concourse.mybir as mybir
    from concourse import tile
    from concourse.tile_rust import add_dep_helper

    F32 = mybir.dt.float32
    BF16 = mybir.dt.bfloat16
    nc = bacc.Bacc("TRN2", target_bir_lowering=False, debug=False, num_devices=CORES)

    cb_d = nc.declare_dram_parameter("cb", [128, NB], BF16, isOutput=False)
    ind_d = nc.declare_dram_parameter("ind8", [128, 200], BF16, isOutput=False)
    # scan/xT split into separate DRAM tensors so each transfer reads
    # CONTIGUOUS DRAM (a column-slice of one big tensor is 16KB-strided
    # 2KB chunks, which halves effective HBM bandwidth)
    scan_ds = [
        nc.declare_dram_parameter(
            f"scan{k}",
            [128, 256 * (SCAN_CUTS[k + 1] - SCAN_CUTS[k])], BF16,
            isOutput=False,
        )
        for k in range(len(SCAN_CUTS) - 1)
    ]
    XT_BOUNDS = [0, 1536, 4096, 6656, XT_FREE]
    xT_ds = [
        nc.declare_dram_parameter(
            f"xT{q}", [96, XT_BOUNDS[q + 1] - XT_BOUNDS[q]], BF16, isOutput=False
        )
        for q in range(4)
    ]
    b2_d = nc.declare_dram_parameter("b2col", [128, 1], F32, isOutput=False)
    out_d = nc.declare_dram_parameter("out", [12, XT_FREE], F32, isOutput=True)

    Gelu = mybir.ActivationFunctionType.Gelu

    with tile.TileContext(nc) as tc:
        with (
            tc.tile_pool(name="const", bufs=1) as cpool,
            tc.tile_pool(name="data", bufs=1) as dpool,
            tc.tile_pool(name="gp", bufs=3) as gpool,
        ):
            warm_sb = cpool.tile([1, 8], F32)
            warmd_sb = cpool.tile([32, 128], BF16)
            cb_sb = cpool.tile([128, NB], BF16)
            ind_sb = cpool.tile([128, 200], BF16)
            b2_sb = cpool.tile([128, 1], F32)
            acc1 = cpool.tile([65, 1], BF16)
            bias_sb = cpool.tile([128, 1], F32)
            s_sb = cpool.tile([38, 256], BF16)
            sT_sb = cpool.tile([128, 12], BF16)

            scan_sb = dpool.tile([128, SCAN_FREE], BF16)
            xT_sb = dpool.tile([96, XT_FREE], BF16)
            stage_sb = dpool.tile([128, XT_FREE], F32)

            # ---- DMA enqueues ----
            # ind + consts first on scalar (needed by stage 1); fp8 scan
            # slices interleave across sync/gpsimd in consumption order; xT
            # quarters wait for the whole scan (keeps the bias critical path
            # at full DMA bandwidth); b2col last (needed at ~first add)
            nc.scalar.dma_start(ind_sb[:], ind_d.ap())
            nc.scalar.dma_start(cb_sb[:], cb_d.ap())
            SLICE_Q = [nc.sync, nc.gpsimd, nc.sync, nc.gpsimd,
                       nc.sync, nc.gpsimd, nc.sync, nc.gpsimd, nc.sync]
            scan_dmas = []
            for k, eng in enumerate(SLICE_Q):
                lo, hi = 256 * SCAN_CUTS[k], 256 * SCAN_CUTS[k + 1]
                scan_dmas.append(eng.dma_start(
                    scan_sb[:, lo:hi], scan_ds[k].ap(),
                ))
            # xT: a small early piece (chunks 0-2) rides gpsimd with no dep
            # so z0-z2 can prefill during the chain; the remaining three
            # pieces wait for the scan (bias critical path owns the HBM)
            XT_PIECE_Q = [nc.gpsimd, nc.sync, nc.gpsimd, nc.sync]
            for q, eng in enumerate(XT_PIECE_Q):
                lo, hi = XT_BOUNDS[q], XT_BOUNDS[q + 1]
                xi = eng.dma_start(xT_sb[:, lo:hi], xT_ds[q].ap())
                if q > 0:
                    for sd in scan_dmas:
                        add_dep_helper(xi.ins, sd.ins, sync=True,
                                       reason="serialize xT behind scan")
            nc.scalar.dma_start(b2_sb[:], b2_d.ap())

            # warm-ups AFTER the dma enqueues so the queues start moving
            # first: ACT warm (gelu table load) + PE warm-up source
            nc.gpsimd.memset(warm_sb[:], 0.0)
            nc.scalar.activation(warm_sb[:], warm_sb[:], Gelu)
            nc.gpsimd.memset(warmd_sb[:], 0.001)
            nc.gpsimd.memset(acc1[64:65, :], 1.0)

            with tc.tile_pool(name="psZ", bufs=2, space="PSUM") as zp:
                with tc.tile_pool(name="psC", bufs=1, space="PSUM") as pchain:
                    # chain PSUM lives in ONE bank: cols 0:256 stage-1
                    # class sums (+ warm-up junk), 256:268 sT(A+B),
                    # 268:280 sT(C), 280:281 acc E/O, 281:282 bias
                    ps_part = pchain.tile([128, 284], F32)

                    # PE pre-warm: open the HAM clock gate before stage 1
                    for _ in range(16):
                        nc.tensor.matmul(
                            ps_part[:, 0:128], warmd_sb[:], warmd_sb[:],
                            start=True, stop=True,
                        )

                    # stage 1: class sums over the replicated scan region.
                    # mixed tiles cycle indicators in quadrant 0 (rows 0-5);
                    # pure class-4 tiles share tile-5's indicator resident in
                    # quadrant 1 (rows 32-37) -> no LDWEIGHTS between them
                    for T in range(SCAN_TILES):
                        pure = PURE_LO <= T <= PURE_HI
                        ind_T = 6 * PURE_LO if pure else 6 * T
                        nc.tensor.matmul(
                            ps_part[32:38, 0:256] if pure else ps_part[0:6, 0:256],
                            ind_sb[:, ind_T:ind_T + 6],
                            scan_sb[:, 256 * T:256 * (T + 1)],
                            start=(T == PURE_LO if pure else T == 0),
                            stop=(T == PURE_HI if pure else T == SCAN_TILES - 1),
                            tile_position=(0, 32) if pure else (0, 0),
                        )

                    # s (38,256) -> sT (128,12) via R-matmuls that also fold
                    # the two quadrant row groups (R[d,d]=R[32+d,d]=1)
                    nc.vector.tensor_copy(s_sb[:], ps_part[0:38, 0:256])
                    for jhi in range(2):
                        nc.tensor.matmul(
                            ps_part[:, 256 + 6 * jhi:256 + 6 * jhi + 6],
                            s_sb[:, 128 * jhi:128 * (jhi + 1)],
                            cb_sb[0:38, R0:R0 + 6],
                            start=True, stop=True,
                        )
                    nc.vector.tensor_copy(sT_sb[:], ps_part[:, 256:268])

                    # acc = sum_k M2_k @ sT[:, k], even k in quadrant 0
                    # (rows 0:32), odd k in quadrant 1 (rows 32:64)
                    for k in range(12):
                        odd = k % 2
                        nc.tensor.matmul(
                            ps_part[32 * odd:32 * odd + 32, 280:281],
                            cb_sb[:, M20 + 32 * k:M20 + 32 * (k + 1)],
                            sT_sb[:, k:k + 1],
                            start=(k < 2), stop=(k >= 10),
                            tile_position=(0, 32 * odd),
                        )
                    nc.vector.tensor_copy(acc1[0:64, :], ps_part[0:64, 280:281])

                    # bias1_eff = W1cat.T@(accE+accO) + (b1cat + gamma@W1cat)
                    # via the widened [65,128] wb2 (rows 0-31 W1, 32-63 W1,
                    # 64 bconst)
                    nc.tensor.matmul(
                        ps_part[:, 281:282], cb_sb[0:65, WB0:WB0 + 128],
                        acc1[:], start=True, stop=True,
                    )
                    nc.vector.tensor_copy(bias_sb[:], ps_part[:, 281:282])

                with tc.tile_pool(name="psO", bufs=2, space="PSUM") as op:
                    for t in range(NCH):
                        sub = SUB if t < NCH - 1 else SUB_L
                        ch = NSUB * sub
                        t0 = SUB * t
                        z = zp.tile([128, CH], F32)
                        for a in range(NSUB):
                            nc.tensor.matmul(
                                z[:, sub * a:sub * (a + 1)],
                                cb_sb[32 * a:32 * (a + 1), W10:W10 + 128],
                                xT_sb[32 * a:32 * (a + 1), t0:t0 + sub],
                                start=True,
                                stop=True,
                                tile_position=(32 * a, 0),
                            )
                        g = gpool.tile([128, CH], BF16)
                        nc.scalar.activation(g[:, 0:ch], z[:, 0:ch], Gelu,
                                             bias=bias_sb[:])
                        o_ps = op.tile([128, SUB], F32)
                        for c in range(NSUB):
                            nc.tensor.matmul(
                                o_ps[32 * c:32 * c + 4, 0:sub],
                                cb_sb[:, W20:W20 + 4],
                                g[:, sub * c:sub * (c + 1)],
                                start=True,
                                stop=True,
                                tile_position=(0, 32 * c),
                            )
                        nc.vector.tensor_scalar_add(
                            stage_sb[:, t0:t0 + sub], o_ps[:, 0:sub], b2_sb[:]
                        )
                        # batched output DMA on the idle gpsimd queue; the
                        # final (small) batch fans one strip to each of the
                        # 3 queues so their completion latencies overlap
                        if t in (4, 9, 13, 16, 17, NCH - 1):
                            lo = {4: 0, 9: 2560, 13: 5120, 16: 7168,
                                  17: 8704, NCH - 1: 9216}[t]
                            hi = t0 + sub
                            engs = ([nc.gpsimd] * 3 if t != NCH - 1
                                    else [nc.sync, nc.gpsimd, nc.scalar])
                            for c in range(NSUB):
                                engs[c].dma_start(
                                    out_d.ap()[4 * c:4 * c + 4, lo:hi],
                                    stage_sb[32 * c:32 * c + 4, lo:hi],
                                )

    nc.compile()
    return nc


def _get_graph():
    global _GRAPH
    if _GRAPH is None:
        _GRAPH = _build_graph()
    return _GRAPH


# ---------------------------------------------------------------- kernel
def kernel(**inputs):
    import ml_dtypes
    from concourse import bass_utils

    data = np.asarray(inputs["data"], np.float32)
    conv_w = np.asarray(inputs["conv_w"], np.float32)
    conv_b = np.asarray(inputs["conv_b"], np.float32)
    dw = np.asarray(inputs["depth_weight"], np.float32)
    f_w1 = np.asarray(inputs["f_w1"], np.float32)
    f_b1 = np.asarray(inputs["f_b1"], np.float32)
    f_w2 = np.asarray(inputs["f_w2"], np.float32)
    f_b2 = np.asarray(inputs["f_b2"], np.float32)
    s_w1 = np.asarray(inputs["s_w1"], np.float32)
    s_b1 = np.asarray(inputs["s_b1"], np.float32)
    s_w2 = np.asarray(inputs["s_w2"], np.float32)
    s_b2 = np.asarray(inputs["s_b2"], np.float32)

    # --- weight-derived host constants (no data-sized work here) ---
    mats = _build_class_mats(conv_w, conv_b, dw)

    W1cat = np.concatenate([f_w1, s_w1], axis=1)          # (32, 128)
    b1cat = np.concatenate([f_b1, s_b1])                  # (128,)
    gamma_tot = np.zeros(D)
    for (p_lo, p_hi, M, gamma) in mats:
        gamma_tot += (p_hi - p_lo + 1) * gamma
    bconst = b1cat.astype(np.float64) + gamma_tot @ W1cat.astype(np.float64)

    W2cat = np.zeros((128, 4), np.float32)
    W2cat[0:64, 0:3] = f_w2
    W2cat[64:128, 3:4] = s_w2
    b2cat = np.concatenate([f_b2, s_b2]).astype(np.float32)
    b2col = np.zeros((128, 1), np.float32)
    for c in range(NSUB):
        b2col[32 * c:32 * c + 4, 0] = b2cat

    # --- packed constants ---
    # ind (fp8): col block 6T+d, row p: node 128T+p in class d
    ind8 = np.zeros((128, 200), np.float32)
    for dcls, (p_lo, p_hi, M, gamma) in enumerate(mats):
        for node in range(p_lo, p_hi + 1):
            T, p = divmod(node, 128)
            ind8[p, 6 * T + dcls] = 1.0
    # cols 192:198 duplicate the pure-class-4 indicator at a distinct
    # address so group C's LDWEIGHTS (new PE quadrant) is not elided
    ind8[:, 192:198] = ind8[:, 6 * PURE_LO:6 * PURE_LO + 6]
    ind8 = np.ascontiguousarray(ind8.astype(ml_dtypes.bfloat16))

    cb = np.zeros((128, NB), np.float32)
    # R fold matrix: quadrant-0 rows 0-5 and quadrant-1 rows 32-37 -> class d
    for dcls in range(6):
        cb[dcls, R0 + dcls] = 1.0
        cb[32 + dcls, R0 + dcls] = 1.0
    # m2 (128, 384): col block k=6*jhi+d : m2[j, 32k+o] = M_d[o, 128*jhi+j]
    for dcls, (p_lo, p_hi, M, gamma) in enumerate(mats):
        Mf = M.astype(np.float32)
        for jhi in range(2):
            k = 6 * jhi + dcls
            cb[:, M20 + 32 * k:M20 + 32 * (k + 1)] = \
                Mf[:, 128 * jhi:128 * (jhi + 1)].T
    # wb2 (65, 128): rows 0-31 W1cat, 32-63 W1cat, 64 bconst
    cb[0:32, WB0:WB0 + 128] = W1cat
    cb[32:64, WB0:WB0 + 128] = W1cat
    cb[64, WB0:WB0 + 128] = bconst.astype(np.float32)
    # w1cat3 (96, 128) and w2 (128, 4)
    cb[0:96, W10:W10 + 128] = np.tile(W1cat, (3, 1))
    cb[:, W20:W20 + 4] = W2cat
    cb = np.ascontiguousarray(cb.astype(ml_dtypes.bfloat16))

    # --- shards ---
    data_flat = data.reshape(N_CELLS, D)

    # replicated scan region (all 4096 parent nodes), bf16, one contiguous
    # array per DMA slice
    scan = (
        data_flat[0:N_GROUPS * 8].reshape(SCAN_TILES, 128, 256).transpose(1, 0, 2)
        .reshape(128, SCAN_FREE).astype(ml_dtypes.bfloat16)
    )
    scan_slices = [
        np.ascontiguousarray(scan[:, 256 * SCAN_CUTS[k]:256 * SCAN_CUTS[k + 1]])
        for k in range(len(SCAN_CUTS) - 1)
    ]
    XT_BOUNDS = [0, 1536, 4096, 6656, XT_FREE]

    in_maps = []
    for i in range(CORES):
        base = LEAF0 + CELLS_MAIN * i
        end = min(base + CELLS_CORE, N_CELLS)
        x_lin = np.zeros((CELLS_CORE, D), np.float32)
        x_lin[0:end - base] = data_flat[base:end]
        xA = (
            x_lin[:(NCH - 1) * CH].reshape(NCH - 1, NSUB, SUB, D)
            .transpose(1, 3, 0, 2).reshape(96, (NCH - 1) * SUB)
        )
        xB = (
            x_lin[(NCH - 1) * CH:].reshape(NSUB, SUB_L, D)
            .transpose(0, 2, 1).reshape(96, SUB_L)
        )
        xT = np.concatenate([xA, xB], axis=1).astype(ml_dtypes.bfloat16)
        im = {"cb": cb, "ind8": ind8, "b2col": b2col}
        for k in range(len(SCAN_CUTS) - 1):
            im[f"scan{k}"] = scan_slices[k]
        for q in range(4):
            im[f"xT{q}"] = np.ascontiguousarray(
                xT[:, XT_BOUNDS[q]:XT_BOUNDS[q + 1]]
            )
        in_maps.append(im)

    nc = _get_graph()
    res = bass_utils.run_bass_kernel_spmd(nc, in_maps, core_ids=list(range(CORES)))

    out_flat = np.zeros((N_CELLS, 4), np.float32)
    for i in range(CORES):
        base = LEAF0 + CELLS_MAIN * i
        k = CELLS_MAIN if i < CORES - 1 else CELLS_MAIN + 1
        # planes (12, 9560): row 4c+o holds cells of chunk t at free 512t+cc
        planes = res.results[i]["out"]
        pA = planes[:, :(NCH - 1) * SUB].reshape(NSUB, 4, NCH - 1, SUB)
        cellsA = pA.transpose(2, 0, 3, 1).reshape((NCH - 1) * CH, 4)
        pB = planes[:, (NCH - 1) * SUB:].reshape(NSUB, 4, SUB_L)
        cellsB = pB.transpose(0, 2, 1).reshape(CH_L, 4)
        cells = np.concatenate([cellsA, cellsB], axis=0)   # (28680, 4)
        out_flat[base:base + k] = cells[:k]
    return out_flat.reshape(N_NODES, 2, 2, 2, 4)


# revision 51
# speedup vs baseline: 1.1579x; 1.1311x over previous
"""Trainium2 Bass kernel for nn_AdExternal_N3Tree (gnn_message_passing).

Key insight: the reference's sequential 32768-step scan only affects the
output through `acc` (a 32-vector). Each parent's 8-child sibling group is an
independent serial chain that is LINEAR in that parent's original 8x32 block,
and group depth is constant within each of 6 contiguous parent-index classes.
So  acc = sum_d M_d @ s_d + gamma_tot,  where s_d is the sum of flattened
parent blocks over class d (a pure 4MB reduction) and M_d (32x256) / gamma
are tiny matrices computed on the host from conv_w/conv_b/depth_weight alone.

The leaf cells that feed the MLPs (flat cells 32767..262143) are never
written by the scan, so  out[leaf] = MLP(data_flat[leaf] + acc)  and cells
0..32766 are zero.

Device work per core (SPMD on 8 cores, no collectives - each core is fully
independent, which also makes the kernel immune to cross-core launch skew):
  - scan DMA as 9 contiguous-DRAM slices interleaved over the sync + gpsimd
    queues (per-slice DRAM tensors double effective HBM bandwidth vs column
    slices of one tensor; the last slice is a single 64KB tile so the bias
    critical path ends on a small, early transfer); consts ride the scalar
    queue ONLY in small volume (bulk scalar-queue DMA provably drops the
    whole-chip clock by 1.2x); xT is dep-blocked behind the scan except a
    small chunk-0-2 piece that prefills z
  - stage 1 pipelined per slice: class sums with two PE quadrants: mixed
    node tiles cycle indicator weights in quadrant 0; the 26 pure-class-4
    tiles reuse ONE resident indicator in quadrant 1 (LDWEIGHTS elision ->
    256-col feed-bound, 213ns/tile)
  - tiny chain in bf16: s -> transpose (R-matrix matmuls fold both quadrant
    row groups) -> even/odd acc quadrants -> widened [65,128] bias matmul
    merges them for free -> folded layer-1 bias (128,)
  - MLP over a 28680-cell slice in bf16: x@W1cat (row-tiled) -> GELU+bias on
    ScalarE (the 1 elem/cycle/lane throughput bound, f32 PSUM in, 1.44us per
    1536-cell chunk) -> @W2cat (col-tiled) -> +b2 evict on DVE; the chain's
    single PSUM bank + outer z-pool lets z0/z1 prefill during the chain
  - output in 5 staged gpsimd batches; the final 512-col batch fans one
    strip to each of the 3 queues so completion latencies overlap.
"""
import sys

for _p in ("/opt/trn_rl_repo", "/opt/trn_rl_repo/concourse"):
    if _p not in sys.path:
        sys.path.insert(0, _p)

import numpy as np

N_NODES = 32768
D = 32
N_GROUPS = 4096
N_CELLS = N_NODES * 8           # 262144
LEAF0 = N_NODES - 1             # 32767 first leaf cell
CORES = 8
CELLS_MAIN = 28672              # cells each core contributes (core 7: +1)
CH = 1536                       # cells per chunk (3 row-tiled sub-chunks of 512)
NCH = 19
SUB = 512
NSUB = 3
SUB_L = 512                     # last-chunk sub size (PSUM writes must stay
                                # bank-aligned, so no odd-size trim)
CH_L = NSUB * SUB_L
CELLS_CORE = (NCH - 1) * CH + CH_L   # 28680 >= 28673
SCAN_TILES = 32                 # replicated scan: 32 node-tiles of 128
SCAN_FREE = SCAN_TILES * 256    # 8192
XT_FREE = (NCH - 1) * SUB + SUB_L    # 9560 (free cols per band row)

# stage-1 tile classification: tiles fully inside class 4 share one
# indicator (nodes 640..3967 = tiles 5..30); the rest are "mixed"
PURE_LO, PURE_HI = 5, 30

# scan DMA slice boundaries in 128-node tiles: the LAST slice is a single
# tile so the bias critical path ends on a small, early-landing transfer
SCAN_CUTS = [0, 4, 8, 12, 16, 20, 24, 28, 31, 32]

# constsB (bf16, [128, NB]) column map (ind lives in its own fp8 tensor)
R0 = 0                          # R fold matrix [38, 6]
M20 = R0 + 6                    # 6: m2 [128, 12*32]
WB0 = M20 + 384                 # 390: wb2 [65, 128]
W10 = WB0 + 128                 # 518: w1cat3 [96, 128]
W20 = W10 + 128                 # 646: w2 [128, 4]
NB = W20 + 4                    # 650

# (p_lo, p_hi_inclusive, conv_depth, n_children, extra_j0_step)
CLASSES = [
    (0, 0, 1, 8, True),
    (1, 8, 2, 8, False),
    (9, 72, 3, 8, False),
    (73, 584, 4, 8, False),
    (585, 4094, 5, 8, False),
    (4095, 4095, 5, 7, False),
]


# ---------------------------------------------------------------- host math
def _chain(conv_w_d, conv_b_d, dw_d, n_children):
    W = conv_w_d.astype(np.float64)
    b = conv_b_d.astype(np.float64)
    Wk = [W[:, :, k] for k in range(8)]
    A, beta = {}, {}
    if n_children == 7:
        A7 = np.zeros((8, D, D))
        A7[7] = np.eye(D)
        A[7] = A7
        beta[7] = np.zeros(D)
        cs = range(6, -1, -1)
    else:
        cs = range(7, -1, -1)
    for c in cs:
        Ac = np.zeros((8, D, D))
        bc = b.copy()
        for k in range(0, c + 1):
            Ac[k] += Wk[k]
        for m in range(c + 1, 8):
            for k in range(8):
                Ac[k] += Wk[m] @ A[m][k]
            bc += Wk[m] @ beta[m]
        A[c] = Ac
        beta[c] = bc
    Msum = np.zeros((8, D, D))
    gamma = np.zeros(D)
    for c in (range(8) if n_children == 8 else range(7)):
        Msum += dw_d * A[c]
        gamma += dw_d * beta[c]
    return A, beta, Msum, gamma


def _build_class_mats(conv_w, conv_b, depth_weight):
    out = []
    for (p_lo, p_hi, dep, nch, extra) in CLASSES:
        A, beta, Msum, gamma = _chain(
            conv_w[dep], conv_b[dep], float(depth_weight[dep]), nch
        )
        if extra:
            W0 = conv_w[0].astype(np.float64)
            b0 = conv_b[0].astype(np.float64)
            W0k = [W0[:, :, k] for k in range(8)]
            Ae = np.zeros((8, D, D))
            be = b0.copy()
            for m in range(8):
                for k in range(8):
                    Ae[k] += W0k[m] @ A[m][k]
                be += W0k[m] @ beta[m]
            Msum = Msum + float(depth_weight[0]) * Ae
            gamma = gamma + float(depth_weight[0]) * be
        M = np.concatenate([Msum[k] for k in range(8)], axis=1)  # (D, 8D)
        out.append((p_lo, p_hi, M, gamma))
    return out


# ---------------------------------------------------------------- device graph
_GRAPH = None


def _build_graph():
    import concourse.bacc as bacc
    import concourse.mybir as mybir
    from concourse import tile
    from concourse.tile_rust import add_dep_helper

    F32 = mybir.dt.float32
    BF16 = mybir.dt.bfloat16
    nc = bacc.Bacc("TRN2", target_bir_lowering=False, debug=False, num_devices=CORES)

    cb_d = nc.declare_dram_parameter("cb", [128, NB], BF16, isOutput=False)
    ind_d = nc.declare_dram_parameter("ind8", [128, 200], BF16, isOutput=False)
    # scan/xT split into separate DRAM tensors so each transfer reads
    # CONTIGUOUS DRAM (a column-slice of one big tensor is 16KB-strided
    # 2KB chunks, which halves effective HBM bandwidth)
    scan_ds = [
        nc.declare_dram_parameter(
            f"scan{k}",
            [128, 256 * (SCAN_CUTS[k + 1] - SCAN_CUTS[k])], BF16,
            isOutput=False,
        )
        for k in range(len(SCAN_CUTS) - 1)
    ]
    XT_BOUNDS = [0, 1536, 4096, 6656, XT_FREE]
    xT_ds = [
        nc.declare_dram_parameter(
            f"xT{q}", [96, XT_BOUNDS[q + 1] - XT_BOUNDS[q]], BF16, isOutput=False
        )
        for q in range(4)
    ]
    b2_d = nc.declare_dram_parameter("b2col", [128, 1], F32, isOutput=False)
    out_d = nc.declare_dram_parameter("out", [12, XT_FREE], F32, isOutput=True)

    Gelu = mybir.ActivationFunctionType.Gelu

    with tile.TileContext(nc) as tc:
        with (
            tc.tile_pool(name="const", bufs=1) as cpool,
            tc.tile_pool(name="data", bufs=1) as dpool,
            tc.tile_pool(name="gp", bufs=3) as gpool,
        ):
            warm_sb = cpool.tile([1, 8], F32)
            warmd_sb = cpool.tile([32, 128], BF16)
            cb_sb = cpool.tile([128, NB], BF16)
            ind_sb = cpool.tile([128, 200], BF16)
            b2_sb = cpool.tile([128, 1], F32)
            acc1 = cpool.tile([65, 1], BF16)
            bias_sb = cpool.tile([128, 1], F32)
            s_sb = cpool.tile([38, 256], BF16)
            sT_sb = cpool.tile([128, 12], BF16)

            scan_sb = dpool.tile([128, SCAN_FREE], BF16)
            xT_sb = dpool.tile([96, XT_FREE], BF16)
            stage_sb = dpool.tile([128, XT_FREE], F32)

            # ---- DMA enqueues ----
            # ind + consts first on scalar (needed by stage 1); fp8 scan
            # slices interleave across sync/gpsimd in consumption order; xT
            # quarters wait for the whole scan (keeps the bias critical path
            # at full DMA bandwidth); b2col last (needed at ~first add)
            nc.scalar.dma_start(ind_sb[:], ind_d.ap())
            nc.scalar.dma_start(cb_sb[:], cb_d.ap())
            SLICE_Q = [nc.sync, nc.gpsimd, nc.sync, nc.gpsimd,
                       nc.sync, nc.gpsimd, nc.sync, nc.gpsimd, nc.sync]
            scan_dmas = []
            for k, eng in enumerate(SLICE_Q):
                lo, hi = 256 * SCAN_CUTS[k], 256 * SCAN_CUTS[k + 1]
                scan_dmas.append(eng.dma_start(
                    scan_sb[:, lo:hi], scan_ds[k].ap(),
                ))
            # xT: a small early piece (chunks 0-2) rides gpsimd with no dep
            # so z0-z2 can prefill during the chain; the remaining three
            # pieces wait for the scan (bias critical path owns the HBM)
            XT_PIECE_Q = [nc.gpsimd, nc.sync, nc.gpsimd, nc.sync]
            for q, eng in enumerate(XT_PIECE_Q):
                lo, hi = XT_BOUNDS[q], XT_BOUNDS[q + 1]
                xi = eng.dma_start(xT_sb[:, lo:hi], xT_ds[q].ap())
                if q > 0:
                    for sd in scan_dmas:
                        add_dep_helper(xi.ins, sd.ins, sync=True,
                                       reason="serialize xT behind scan")
            nc.scalar.dma_start(b2_sb[:], b2_d.ap())

            # warm-ups AFTER the dma enqueues so the queues start moving
            # first: ACT warm (gelu table load) + PE warm-up source
            nc.gpsimd.memset(warm_sb[:], 0.0)
            nc.scalar.activation(warm_sb[:], warm_sb[:], Gelu)
            nc.gpsimd.memset(warmd_sb[:], 0.001)
            nc.gpsimd.memset(acc1[64:65, :], 1.0)

            with tc.tile_pool(name="psZ", bufs=2, space="PSUM") as zp:
                with tc.tile_pool(name="psC", bufs=1, space="PSUM") as pchain:
                    # chain PSUM lives in ONE bank: cols 0:256 stage-1
                    # class sums (+ warm-up junk), 256:268 sT(A+B),
                    # 268:280 sT(C), 280:281 acc E/O, 281:282 bias
                    ps_part = pchain.tile([128, 284], F32)

                    # PE pre-warm: open the HAM clock gate before stage 1
                    for _ in range(16):
                        nc.tensor.matmul(
                            ps_part[:, 0:128], warmd_sb[:], warmd_sb[:],
                            start=True, stop=True,
                        )

                    # stage 1: class sums over the replicated scan region.
                    # mixed tiles cycle indicators in quadrant 0 (rows 0-5);
                    # pure class-4 tiles share tile-5's indicator resident in
                    # quadrant 1 (rows 32-37) -> no LDWEIGHTS between them
                    for T in range(SCAN_TILES):
                        pure = PURE_LO <= T <= PURE_HI
                        ind_T = 6 * PURE_LO if pure else 6 * T
                        nc.tensor.matmul(
                            ps_part[32:38, 0:256] if pure else ps_part[0:6, 0:256],
                            ind_sb[:, ind_T:ind_T + 6],
                            scan_sb[:, 256 * T:256 * (T + 1)],
                            start=(T == PURE_LO if pure else T == 0),
                            stop=(T == PURE_HI if pure else T == SCAN_TILES - 1),
                            tile_position=(0, 32) if pure else (0, 0),
                        )

                    # s (38,256) -> sT (128,12) via R-matmuls that also fold
                    # the two quadrant row groups (R[d,d]=R[32+d,d]=1)
                    nc.vector.tensor_copy(s_sb[:], ps_part[0:38, 0:256])
                    for jhi in range(2):
                        nc.tensor.matmul(
                            ps_part[:, 256 + 6 * jhi:256 + 6 * jhi + 6],
                            s_sb[:, 128 * jhi:128 * (jhi + 1)],
                            cb_sb[0:38, R0:R0 + 6],
                            start=True, stop=True,
                        )
                    nc.vector.tensor_copy(sT_sb[:], ps_part[:, 256:268])

                    # acc = sum_k M2_k @ sT[:, k], even k in quadrant 0
                    # (rows 0:32), odd k in quadrant 1 (rows 32:64)
                    for k in range(12):
                        odd = k % 2
                        nc.tensor.matmul(
                            ps_part[32 * odd:32 * odd + 32, 280:281],
                            cb_sb[:, M20 + 32 * k:M20 + 32 * (k + 1)],
                            sT_sb[:, k:k + 1],
                            start=(k < 2), stop=(k >= 10),
                            tile_position=(0, 32 * odd),
                        )
                    nc.vector.tensor_copy(acc1[0:64, :], ps_part[0:64, 280:281])

                    # bias1_eff = W1cat.T@(accE+accO) + (b1cat + gamma@W1cat)
                    # via the widened [65,128] wb2 (rows 0-31 W1, 32-63 W1,
                    # 64 bconst)
                    nc.tensor.matmul(
                        ps_part[:, 281:282], cb_sb[0:65, WB0:WB0 + 128],
                        acc1[:], start=True, stop=True,
                    )
                    nc.vector.tensor_copy(bias_sb[:], ps_part[:, 281:282])

                with tc.tile_pool(name="psO", bufs=2, space="PSUM") as op:
                    for t in range(NCH):
                        last = t == NCH - 1
                        t0 = SUB * t
                        # the final chunk only needs 1025 of its 1536 cells;
                        # narrow its THIRD sub to 8 columns (PSUM writes stay
                        # bank-aligned at col 1024) to trim the GELU floor
                        subs = [SUB, SUB, 8] if last else [SUB] * NSUB
                        ch = sum(subs)
                        z = zp.tile([128, CH], F32)
                        for a in range(NSUB):
                            nc.tensor.matmul(
                                z[:, SUB * a:SUB * a + subs[a]],
                                cb_sb[32 * a:32 * (a + 1), W10:W10 + 128],
                                xT_sb[32 * a:32 * (a + 1), t0:t0 + subs[a]],
                                start=True,
                                stop=True,
                                tile_position=(32 * a, 0),
                            )
                        g = gpool.tile([128, CH], BF16)
                        nc.scalar.activation(g[:, 0:ch], z[:, 0:ch], Gelu,
                                             bias=bias_sb[:])
                        o_ps = op.tile([128, SUB], F32)
                        for c in range(NSUB):
                            nc.tensor.matmul(
                                o_ps[32 * c:32 * c + 4, 0:subs[c]],
                                cb_sb[:, W20:W20 + 4],
                                g[:, SUB * c:SUB * c + subs[c]],
                                start=True,
                                stop=True,
                                tile_position=(0, 32 * c),
                            )
                        nc.vector.tensor_scalar_add(
                            stage_sb[:, t0:t0 + SUB], o_ps[:, 0:SUB], b2_sb[:]
                        )
                        # batched output DMA on the idle gpsimd queue; the
                        # final (small) batch fans one strip to each of the
                        # 3 queues so their completion latencies overlap
                        if t in (4, 9, 13, 16, 17, NCH - 1):
                            lo = {4: 0, 9: 2560, 13: 5120, 16: 7168,
                                  17: 8704, NCH - 1: 9216}[t]
                            hi = t0 + SUB
                            engs = ([nc.gpsimd] * 3 if t != NCH - 1
                                    else [nc.sync, nc.gpsimd, nc.scalar])
                            for c in range(NSUB):
                                engs[c].dma_start(
                                    out_d.ap()[4 * c:4 * c + 4, lo:hi],
                                    stage_sb[32 * c:32 * c + 4, lo:hi],
                                )

    nc.compile()
    return nc


def _get_graph():
    global _GRAPH
    if _GRAPH is None:
        _GRAPH = _build_graph()
    return _GRAPH


# ---------------------------------------------------------------- kernel
def kernel(**inputs):
    import ml_dtypes
    from concourse import bass_utils

    data = np.asarray(inputs["data"], np.float32)
    conv_w = np.asarray(inputs["conv_w"], np.float32)
    conv_b = np.asarray(inputs["conv_b"], np.float32)
    dw = np.asarray(inputs["depth_weight"], np.float32)
    f_w1 = np.asarray(inputs["f_w1"], np.float32)
    f_b1 = np.asarray(inputs["f_b1"], np.float32)
    f_w2 = np.asarray(inputs["f_w2"], np.float32)
    f_b2 = np.asarray(inputs["f_b2"], np.float32)
    s_w1 = np.asarray(inputs["s_w1"], np.float32)
    s_b1 = np.asarray(inputs["s_b1"], np.float32)
    s_w2 = np.asarray(inputs["s_w2"], np.float32)
    s_b2 = np.asarray(inputs["s_b2"], np.float32)

    # --- weight-derived host constants (no data-sized work here) ---
    mats = _build_class_mats(conv_w, conv_b, dw)

    W1cat = np.concatenate([f_w1, s_w1], axis=1)          # (32, 128)
    b1cat = np.concatenate([f_b1, s_b1])                  # (128,)
    gamma_tot = np.zeros(D)
    for (p_lo, p_hi, M, gamma) in mats:
        gamma_tot += (p_hi - p_lo + 1) * gamma
    bconst = b1cat.astype(np.float64) + gamma_tot @ W1cat.astype(np.float64)

    W2cat = np.zeros((128, 4), np.float32)
    W2cat[0:64, 0:3] = f_w2
    W2cat[64:128, 3:4] = s_w2
    b2cat = np.concatenate([f_b2, s_b2]).astype(np.float32)
    b2col = np.zeros((128, 1), np.float32)
    for c in range(NSUB):
        b2col[32 * c:32 * c + 4, 0] = b2cat

    # --- packed constants ---
    # ind (fp8): col block 6T+d, row p: node 128T+p in class d
    ind8 = np.zeros((128, 200), np.float32)
    for dcls, (p_lo, p_hi, M, gamma) in enumerate(mats):
        for node in range(p_lo, p_hi + 1):
            T, p = divmod(node, 128)
            ind8[p, 6 * T + dcls] = 1.0
    # cols 192:198 duplicate the pure-class-4 indicator at a distinct
    # address so group C's LDWEIGHTS (new PE quadrant) is not elided
    ind8[:, 192:198] = ind8[:, 6 * PURE_LO:6 * PURE_LO + 6]
    ind8 = np.ascontiguousarray(ind8.astype(ml_dtypes.bfloat16))

    cb = np.zeros((128, NB), np.float32)
    # R fold matrix: quadrant-0 rows 0-5 and quadrant-1 rows 32-37 -> class d
    for dcls in range(6):
        cb[dcls, R0 + dcls] = 1.0
        cb[32 + dcls, R0 + dcls] = 1.0
    # m2 (128, 384): col block k=6*jhi+d : m2[j, 32k+o] = M_d[o, 128*jhi+j]
    for dcls, (p_lo, p_hi, M, gamma) in enumerate(mats):
        Mf = M.astype(np.float32)
        for jhi in range(2):
            k = 6 * jhi + dcls
            cb[:, M20 + 32 * k:M20 + 32 * (k + 1)] = \
                Mf[:, 128 * jhi:128 * (jhi + 1)].T
    # wb2 (65, 128): rows 0-31 W1cat, 32-63 W1cat, 64 bconst
    cb[0:32, WB0:WB0 + 128] = W1cat
    cb[32:64, WB0:WB0 + 128] = W1cat
    cb[64, WB0:WB0 + 128] = bconst.astype(np.float32)
    # w1cat3 (96, 128) and w2 (128, 4)
    cb[0:96, W10:W10 + 128] = np.tile(W1cat, (3, 1))
    cb[:, W20:W20 + 4] = W2cat
    cb = np.ascontiguousarray(cb.astype(ml_dtypes.bfloat16))

    # --- shards ---
    data_flat = data.reshape(N_CELLS, D)

    # replicated scan region (all 4096 parent nodes), bf16, one contiguous
    # array per DMA slice
    scan = (
        data_flat[0:N_GROUPS * 8].reshape(SCAN_TILES, 128, 256).transpose(1, 0, 2)
        .reshape(128, SCAN_FREE).astype(ml_dtypes.bfloat16)
    )
    scan_slices = [
        np.ascontiguousarray(scan[:, 256 * SCAN_CUTS[k]:256 * SCAN_CUTS[k + 1]])
        for k in range(len(SCAN_CUTS) - 1)
    ]
    XT_BOUNDS = [0, 1536, 4096, 6656, XT_FREE]

    in_maps = []
    for i in range(CORES):
        base = LEAF0 + CELLS_MAIN * i
        end = min(base + CELLS_CORE, N_CELLS)
        x_lin = np.zeros((CELLS_CORE, D), np.float32)
        x_lin[0:end - base] = data_flat[base:end]
        xA = (
            x_lin[:(NCH - 1) * CH].reshape(NCH - 1, NSUB, SUB, D)
            .transpose(1, 3, 0, 2).reshape(96, (NCH - 1) * SUB)
        )
        xB = (
            x_lin[(NCH - 1) * CH:].reshape(NSUB, SUB_L, D)
            .transpose(0, 2, 1).reshape(96, SUB_L)
        )
        xT = np.concatenate([xA, xB], axis=1).astype(ml_dtypes.bfloat16)
        im = {"cb": cb, "ind8": ind8, "b2col": b2col}
        for k in range(len(SCAN_CUTS) - 1):
            im[f"scan{k}"] = scan_slices[k]
        for q in range(4):
            im[f"xT{q}"] = np.ascontiguousarray(
                xT[:, XT_BOUNDS[q]:XT_BOUNDS[q + 1]]
            )
        in_maps.append(im)

    nc = _get_graph()
    res = bass_utils.run_bass_kernel_spmd(nc, in_maps, core_ids=list(range(CORES)))

    out_flat = np.zeros((N_CELLS, 4), np.float32)
    for i in range(CORES):
        base = LEAF0 + CELLS_MAIN * i
        k = CELLS_MAIN if i < CORES - 1 else CELLS_MAIN + 1
        # planes (12, 9560): row 4c+o holds cells of chunk t at free 512t+cc
        planes = res.results[i]["out"]
        pA = planes[:, :(NCH - 1) * SUB].reshape(NSUB, 4, NCH - 1, SUB)
        cellsA = pA.transpose(2, 0, 3, 1).reshape((NCH - 1) * CH, 4)
        pB = planes[:, (NCH - 1) * SUB:].reshape(NSUB, 4, SUB_L)
        cellsB = pB.transpose(0, 2, 1).reshape(CH_L, 4)
        cells = np.concatenate([cellsA, cellsB], axis=0)   # (28680, 4)
        out_flat[base:base + k] = cells[:k]
    return out_flat.reshape(N_NODES, 2, 2, 2, 4)
